# revision 2
# baseline (speedup 1.0000x reference)
"""Causal dot-product attention (B=4, S=2048, D=1024, single head) on 8 TRN2 cores.

Sharding: core c = (batch c//2, q-tile parity c%2). Each core computes the
projections for its batch and attention for its 8 query tiles of 128 rows
(odd or even global q-tiles, ordered by descending causal key-need), so the
per-slot key-range bounds BETA are identical across cores (SPMD-uniform).

Numerics: Q/K projections and the score matmul run in true fp32 (4 cyc/row on
the PE).  The V projection and probs@V matmul run in fp32r (11-bit mantissa,
1 cyc/row) - their error is ~5e-4 relative on `out`, far below fp32 score
sensitivity.  Scores are scaled by 1/sqrt(d) inside the QxT projection copy.
"""

import math
import os
from contextlib import ExitStack

import numpy as np

B, S, D = 4, 2048, 1024
NB = D // 128          # 8 contraction chunks
KT = S // 128          # 16 key tiles
NSLOT = 8              # query tiles per core
QL = NSLOT * 128       # local query rows per core (1024)
SCALE = 1.0 / math.sqrt(D)
NEG = -1.0e30

CAUSAL = os.environ.get("KB_CAUSAL", "1") == "1"
USE_FP32R = os.environ.get("KB_FP32R", "1") == "1"
BETA = [16, 14, 12, 10, 8, 6, 4, 2] if CAUSAL else [16] * 8   # k-tiles per slot

_CACHE = {}
LAST_RESULTS = None


def _round_fp32r(a: np.ndarray) -> np.ndarray:
    """Round-to-nearest-even to 11 mantissa bits (the fp32r grid)."""
    b = np.ascontiguousarray(a, dtype=np.float32).view(np.uint32)
    r = (b + np.uint32(0x7FF) + ((b >> np.uint32(12)) & np.uint32(1))) & np.uint32(0xFFFFF000)
    return r.view(np.float32)


def _build_nc():
    import concourse.tile as tile
    from concourse import bacc, mybir
    from concourse.masks import make_identity

    f32 = mybir.dt.float32
    f32r = mybir.dt.float32r
    vdt = f32r if USE_FP32R else f32

    nc = bacc.Bacc("TRN2", target_bir_lowering=False, debug=False)

    xT = nc.dram_tensor("xT", [D, S], f32, kind="ExternalInput")       # xf[b].T
    xqT = nc.dram_tensor("xqT", [D, QL], f32, kind="ExternalInput")    # query cols of xf[b].T
    xvT = nc.dram_tensor("xvT", [D, S], vdt, kind="ExternalInput")     # x[b].T (rounded)
    Qw = nc.dram_tensor("Qw", [D, D], f32, kind="ExternalInput")
    Kw = nc.dram_tensor("Kw", [D, D], f32, kind="ExternalInput")
    Vw = nc.dram_tensor("Vw", [D, D], vdt, kind="ExternalInput")       # rounded
    maskT = nc.dram_tensor("maskT", [QL, S], f32, kind="ExternalInput")
    probs_o = nc.dram_tensor("probs_o", [QL, S], f32, kind="ExternalOutput")
    out_o = nc.dram_tensor("out_o", [QL, D], f32, kind="ExternalOutput")

    Exp = mybir.ActivationFunctionType.Exp
    AX = mybir.AxisListType.X

    with tile.TileContext(nc) as tc, ExitStack() as ctx:
        const_pool = ctx.enter_context(tc.tile_pool(name="const", bufs=1))
        ident = const_pool.tile([128, 128], f32)
        make_identity(nc, ident[:])

        kxt_pool = ctx.enter_context(tc.tile_pool(name="kxt", bufs=1))
        kxt = kxt_pool.tile([128, NB * S], f32)          # [d' in chunk, dc*S + k]
        vx_pool = ctx.enter_context(tc.tile_pool(name="vx", bufs=1))
        vx = vx_pool.tile([128, KT * D], vdt)            # [k in chunk, kc*D + d']
        qxtd_pool = ctx.enter_context(tc.tile_pool(name="qxtd", bufs=1, space="DRAM"))
        qxt_d = qxtd_pool.tile([D, QL], f32)

        # ---- P1: QxT = (xq @ Q).T * SCALE  -> DRAM spill -------------------
        with tc.tile_pool(name="p1sb", bufs=1) as p1sb, \
             tc.tile_pool(name="p1st", bufs=2) as p1st, \
             tc.tile_pool(name="p1ps", bufs=4, space="PSUM") as p1ps:
            xq_all = p1sb.tile([128, NB * QL], f32)
            q_all = p1sb.tile([128, NB * D], f32)
            for dc in range(NB):
                nc.sync.dma_start(xq_all[:, dc * QL:(dc + 1) * QL],
                                  xqT.ap()[dc * 128:(dc + 1) * 128, :])
                nc.sync.dma_start(q_all[:, dc * D:(dc + 1) * D],
                                  Qw.ap()[dc * 128:(dc + 1) * 128, :])
            for dt_ in range(NB):
                stage = p1st.tile([128, QL], f32)
                for qs in range(QL // 512):
                    ps = p1ps.tile([128, 512], f32)
                    for dc in range(NB):
                        nc.tensor.matmul(
                            ps[:],
                            q_all[:, dc * D + dt_ * 128: dc * D + (dt_ + 1) * 128],
                            xq_all[:, dc * QL + qs * 512: dc * QL + (qs + 1) * 512],
                            start=(dc == 0), stop=(dc == NB - 1))
                    nc.scalar.mul(stage[:, qs * 512:(qs + 1) * 512], ps[:], SCALE)
                nc.sync.dma_start(qxt_d[dt_ * 128:(dt_ + 1) * 128, :], stage[:])

        # ---- P2: KxT = (x @ K).T  (fp32, resident) -------------------------
        with tc.tile_pool(name="p2k", bufs=1) as p2k, \
             tc.tile_pool(name="p2x", bufs=2) as p2x, \
             tc.tile_pool(name="p2ps", bufs=4, space="PSUM") as p2ps:
            k_all = p2k.tile([128, NB * D], f32)
            for dc in range(NB):
                nc.sync.dma_start(k_all[:, dc * D:(dc + 1) * D],
                                  Kw.ap()[dc * 128:(dc + 1) * 128, :])
            for ks in range(S // 512):
                xts = p2x.tile([128, NB * 512], f32)
                for dc in range(NB):
                    nc.sync.dma_start(xts[:, dc * 512:(dc + 1) * 512],
                                      xT.ap()[dc * 128:(dc + 1) * 128, ks * 512:(ks + 1) * 512])
                for dt_ in range(NB):
                    ps = p2ps.tile([128, 512], f32)
                    for dc in range(NB):
                        nc.tensor.matmul(
                            ps[:],
                            k_all[:, dc * D + dt_ * 128: dc * D + (dt_ + 1) * 128],
                            xts[:, dc * 512:(dc + 1) * 512],
                            start=(dc == 0), stop=(dc == NB - 1))
                    nc.scalar.copy(kxt[:, dt_ * S + ks * 512: dt_ * S + (ks + 1) * 512], ps[:])

        # ---- P3: Vx = x @ V  (fp32r, resident) -----------------------------
        with tc.tile_pool(name="p3v", bufs=1) as p3v, \
             tc.tile_pool(name="p3x", bufs=2) as p3x, \
             tc.tile_pool(name="p3ps", bufs=4, space="PSUM") as p3ps:
            v_all = p3v.tile([128, NB * D], vdt)
            for dc in range(NB):
                nc.sync.dma_start(v_all[:, dc * D:(dc + 1) * D],
                                  Vw.ap()[dc * 128:(dc + 1) * 128, :])
            for ks in range(S // 512):
                xvs = p3x.tile([128, NB * 512], vdt)
                for dc in range(NB):
                    nc.sync.dma_start(xvs[:, dc * 512:(dc + 1) * 512],
                                      xvT.ap()[dc * 128:(dc + 1) * 128, ks * 512:(ks + 1) * 512])
                for ktl in range(4):
                    kc = ks * 4 + ktl
                    for dsl in range(2):
                        ps = p3ps.tile([128, 512], f32)
                        for dc in range(NB):
                            nc.tensor.matmul(
                                ps[:],
                                xvs[:, dc * 512 + ktl * 128: dc * 512 + (ktl + 1) * 128],
                                v_all[:, dc * D + dsl * 512: dc * D + (dsl + 1) * 512],
                                start=(dc == 0), stop=(dc == NB - 1))
                        nc.scalar.copy(vx[:, kc * D + dsl * 512: kc * D + (dsl + 1) * 512], ps[:])

        # ---- Attention, one slot (128 queries) at a time -------------------
        with tc.tile_pool(name="aq", bufs=2) as aq, \
             tc.tile_pool(name="am", bufs=1) as am, \
             tc.tile_pool(name="asc", bufs=2) as asc, \
             tc.tile_pool(name="apt", bufs=2) as apt, \
             tc.tile_pool(name="aou", bufs=1) as aou, \
             tc.tile_pool(name="ast", bufs=4) as ast, \
             tc.tile_pool(name="aps", bufs=3, space="PSUM") as aps, \
             tc.tile_pool(name="atp", bufs=2, space="PSUM") as atp, \
             tc.tile_pool(name="aop", bufs=2, space="PSUM") as aop:
            for i in range(NSLOT):
                nk = BETA[i]
                kw = nk * 128
                qxt_s = aq.tile([128, NB * 128], f32)
                for dc in range(NB):
                    nc.sync.dma_start(qxt_s[:, dc * 128:(dc + 1) * 128],
                                      qxt_d[dc * 128:(dc + 1) * 128, i * 128:(i + 1) * 128])
                mask_s = am.tile([128, S], f32)
                nc.sync.dma_start(mask_s[:, :kw], maskT.ap()[i * 128:(i + 1) * 128, :kw])

                scores = asc.tile([128, S], f32)
                off = 0
                while off < kw:
                    w = min(512, kw - off)
                    ps = aps.tile([128, 512], f32)
                    for dc in range(NB):
                        nc.tensor.matmul(
                            ps[:, :w],
                            qxt_s[:, dc * 128:(dc + 1) * 128],
                            kxt[:, dc * S + off: dc * S + off + w],
                            start=(dc == 0), stop=(dc == NB - 1))
                    nc.vector.tensor_add(scores[:, off:off + w], ps[:, :w], mask_s[:, off:off + w])
                    off += w

                negm = ast.tile([128, 1], f32)
                nc.vector.reduce_max(negm[:], scores[:, :kw], axis=AX, negate=True)
                probs = scores  # exp in place
                denom = ast.tile([128, 1], f32)
                nc.scalar.activation(probs[:, :kw], scores[:, :kw], Exp,
                                     bias=negm[:], scale=1.0, accum_out=denom[:])
                rec = ast.tile([128, 1], f32)
                nc.vector.reciprocal(rec[:], denom[:])
                nc.vector.tensor_scalar_mul(probs[:, :kw], probs[:, :kw], rec[:])
                nc.sync.dma_start(probs_o.ap()[i * 128:(i + 1) * 128, :kw], probs[:, :kw])

                pT = apt.tile([128, S], vdt)
                for kc in range(nk):
                    tps = atp.tile([128, 128], f32)
                    nc.tensor.transpose(tps[:], probs[:, kc * 128:(kc + 1) * 128], ident[:])
                    nc.vector.tensor_copy(pT[:, kc * 128:(kc + 1) * 128], tps[:])

                outsb = aou.tile([128, D], f32)
                for dsl in range(2):
                    ps = aop.tile([128, 512], f32)
                    for kc in range(nk):
                        nc.tensor.matmul(
                            ps[:],
                            pT[:, kc * 128:(kc + 1) * 128],
                            vx[:, kc * D + dsl * 512: kc * D + (dsl + 1) * 512],
                            start=(kc == 0), stop=(kc == nk - 1))
                    nc.scalar.copy(outsb[:, dsl * 512:(dsl + 1) * 512], ps[:])
                nc.sync.dma_start(out_o.ap()[i * 128:(i + 1) * 128, :], outsb[:])

    nc.compile()
    return nc


def _qtiles(par: int):
    """Global q-tile indices for parity `par`, descending causal need."""
    return [t for t in range(KT - 1, -1, -1) if t % 2 == par]


def kernel(x, Q, K, V, F):
    global LAST_RESULTS
    from concourse.bass_utils import run_bass_kernel_spmd

    x = np.ascontiguousarray(x, dtype=np.float32)
    Q = np.ascontiguousarray(Q, dtype=np.float32)
    K = np.ascontiguousarray(K, dtype=np.float32)
    V = np.ascontiguousarray(V, dtype=np.float32)
    F = np.ascontiguousarray(F, dtype=np.float32)

    if np.array_equal(F, np.eye(D, dtype=np.float32)):
        xf = x
    else:
        xf = np.matmul(x, F)

    if "nc" not in _CACHE:
        _CACHE["nc"] = _build_nc()
    nc = _CACHE["nc"]

    Vr = _round_fp32r(V) if USE_FP32R else V

    # host-side per-core tensors
    xT_b = [np.ascontiguousarray(xf[b].T) for b in range(B)]
    xvT_b = [_round_fp32r(x[b].T) if USE_FP32R else np.ascontiguousarray(x[b].T)
             for b in range(B)]

    kpos = np.arange(S, dtype=np.int64)
    masks, qsels = {}, {}
    for par in (0, 1):
        tiles = _qtiles(par)
        qpos = np.concatenate([np.arange(t * 128, (t + 1) * 128, dtype=np.int64)
                               for t in tiles])
        masks[par] = np.where(kpos[None, :] <= qpos[:, None],
                              np.float32(0.0), np.float32(NEG)).astype(np.float32)
        qsels[par] = qpos

    in_maps = []
    for c in range(8):
        b, par = c // 2, c % 2
        in_maps.append({
            "xT": xT_b[b],
            "xqT": np.ascontiguousarray(xT_b[b][:, qsels[par]]),
            "xvT": xvT_b[b],
            "Qw": Q, "Kw": K, "Vw": Vr,
            "maskT": masks[par],
        })

    res = run_bass_kernel_spmd(nc, in_maps, list(range(8)))
    LAST_RESULTS = res

    probs = np.zeros((B, S, S), dtype=np.float32)
    out = np.zeros((B, S, D), dtype=np.float32)
    for c in range(8):
        b, par = c // 2, c % 2
        tiles = _qtiles(par)
        pc = res.results[c]["probs_o"]
        oc = res.results[c]["out_o"]
        for i, t in enumerate(tiles):
            probs[b, t * 128:(t + 1) * 128, :] = pc[i * 128:(i + 1) * 128, :]
            out[b, t * 128:(t + 1) * 128, :] = oc[i * 128:(i + 1) * 128, :]
    return (out, probs)


# revision 3
# speedup vs baseline: 1.0932x; 1.0932x over previous
"""Causal dot-product attention (B=4, S=2048, D=1024, single head) on 8 TRN2 cores.

Sharding: core c = (batch c//2, q-tile parity c%2). Each core computes the
projections for its batch and attention for its 8 query tiles of 128 rows
(odd or even global q-tiles, ordered by descending causal key-need), so the
per-slot key-range bounds BETA are identical across cores (SPMD-uniform).

Numerics: Q/K projections and the score matmul run in true fp32 (4 cyc/row on
the PE).  The V projection and probs@V matmul run in fp32r (11-bit mantissa,
1 cyc/row) - their error is ~5e-4 relative on `out`, far below fp32 score
sensitivity.  Scores are scaled by 1/sqrt(d) inside the QxT projection copy.
"""

import math
import os
from contextlib import ExitStack

import numpy as np

B, S, D = 4, 2048, 1024
NB = D // 128          # 8 contraction chunks
KT = S // 128          # 16 key tiles
NSLOT = 8              # query tiles per core
QL = NSLOT * 128       # local query rows per core (1024)
SCALE = 1.0 / math.sqrt(D)
NEG = -1.0e30

CAUSAL = os.environ.get("KB_CAUSAL", "1") == "1"
USE_FP32R = os.environ.get("KB_FP32R", "1") == "1"
BETA = [16, 14, 12, 10, 8, 6, 4, 2] if CAUSAL else [16] * 8   # k-tiles per slot

_CACHE = {}
LAST_RESULTS = None


def _round_fp32r(a: np.ndarray) -> np.ndarray:
    """Round-to-nearest-even to 11 mantissa bits (the fp32r grid)."""
    b = np.ascontiguousarray(a, dtype=np.float32).view(np.uint32)
    r = (b + np.uint32(0x7FF) + ((b >> np.uint32(12)) & np.uint32(1))) & np.uint32(0xFFFFF000)
    return r.view(np.float32)


def _build_nc():
    import concourse.tile as tile
    from concourse import bacc, mybir
    from concourse.masks import make_identity

    f32 = mybir.dt.float32
    f32r = mybir.dt.float32r
    vdt = f32r if USE_FP32R else f32

    nc = bacc.Bacc("TRN2", target_bir_lowering=False, debug=False)

    xT = nc.dram_tensor("xT", [D, S], f32, kind="ExternalInput")       # xf[b].T
    xqT = nc.dram_tensor("xqT", [D, QL], f32, kind="ExternalInput")    # query cols of xf[b].T
    xvT = nc.dram_tensor("xvT", [D, S], vdt, kind="ExternalInput")     # x[b].T (rounded)
    Qw = nc.dram_tensor("Qw", [D, D], f32, kind="ExternalInput")
    Kw = nc.dram_tensor("Kw", [D, D], f32, kind="ExternalInput")
    Vw = nc.dram_tensor("Vw", [D, D], vdt, kind="ExternalInput")       # rounded
    maskT = nc.dram_tensor("maskT", [QL, S], f32, kind="ExternalInput")
    probs_o = nc.dram_tensor("probs_o", [QL, S], f32, kind="ExternalOutput")
    out_o = nc.dram_tensor("out_o", [QL, D], f32, kind="ExternalOutput")

    Exp = mybir.ActivationFunctionType.Exp
    AX = mybir.AxisListType.X

    with tile.TileContext(nc) as tc, ExitStack() as ctx:
        const_pool = ctx.enter_context(tc.tile_pool(name="const", bufs=1))
        ident = const_pool.tile([128, 128], f32)
        make_identity(nc, ident[:])

        kxt_pool = ctx.enter_context(tc.tile_pool(name="kxt", bufs=1))
        kxt = kxt_pool.tile([128, NB * S], f32)          # [d' in chunk, dc*S + k]
        vx_pool = ctx.enter_context(tc.tile_pool(name="vx", bufs=1))
        vx = vx_pool.tile([128, KT * D], vdt)            # [k in chunk, kc*D + d']
        qxtd_pool = ctx.enter_context(tc.tile_pool(name="qxtd", bufs=1, space="DRAM"))
        qxt_d = qxtd_pool.tile([D, QL], f32)

        # ---- P1: QxT = (xq @ Q).T * SCALE  -> DRAM spill -------------------
        with tc.tile_pool(name="p1sb", bufs=1) as p1sb, \
             tc.tile_pool(name="p1st", bufs=2) as p1st, \
             tc.tile_pool(name="p1ps", bufs=4, space="PSUM") as p1ps:
            xq_all = p1sb.tile([128, NB * QL], f32)
            q_all = p1sb.tile([128, NB * D], f32)
            for dc in range(NB):
                nc.sync.dma_start(xq_all[:, dc * QL:(dc + 1) * QL],
                                  xqT.ap()[dc * 128:(dc + 1) * 128, :])
                nc.sync.dma_start(q_all[:, dc * D:(dc + 1) * D],
                                  Qw.ap()[dc * 128:(dc + 1) * 128, :])
            for dt_ in range(NB):
                stage = p1st.tile([128, QL], f32)
                for qs in range(QL // 512):
                    ps = p1ps.tile([128, 512], f32)
                    for dc in range(NB):
                        nc.tensor.matmul(
                            ps[:],
                            q_all[:, dc * D + dt_ * 128: dc * D + (dt_ + 1) * 128],
                            xq_all[:, dc * QL + qs * 512: dc * QL + (qs + 1) * 512],
                            start=(dc == 0), stop=(dc == NB - 1))
                    nc.scalar.mul(stage[:, qs * 512:(qs + 1) * 512], ps[:], SCALE)
                nc.sync.dma_start(qxt_d[dt_ * 128:(dt_ + 1) * 128, :], stage[:])

        # ---- P2: KxT = (x @ K).T  (fp32, resident) -------------------------
        with tc.tile_pool(name="p2k", bufs=1) as p2k, \
             tc.tile_pool(name="p2x", bufs=2) as p2x, \
             tc.tile_pool(name="p2ps", bufs=4, space="PSUM") as p2ps:
            k_all = p2k.tile([128, NB * D], f32)
            for dc in range(NB):
                nc.sync.dma_start(k_all[:, dc * D:(dc + 1) * D],
                                  Kw.ap()[dc * 128:(dc + 1) * 128, :])
            for ks in range(S // 512):
                xts = p2x.tile([128, NB * 512], f32)
                for dc in range(NB):
                    nc.sync.dma_start(xts[:, dc * 512:(dc + 1) * 512],
                                      xT.ap()[dc * 128:(dc + 1) * 128, ks * 512:(ks + 1) * 512])
                for dt_ in range(NB):
                    ps = p2ps.tile([128, 512], f32)
                    for dc in range(NB):
                        nc.tensor.matmul(
                            ps[:],
                            k_all[:, dc * D + dt_ * 128: dc * D + (dt_ + 1) * 128],
                            xts[:, dc * 512:(dc + 1) * 512],
                            start=(dc == 0), stop=(dc == NB - 1))
                    nc.scalar.copy(kxt[:, dt_ * S + ks * 512: dt_ * S + (ks + 1) * 512], ps[:])

        # ---- P3: Vx = x @ V  (fp32r, resident) -----------------------------
        with tc.tile_pool(name="p3v", bufs=1) as p3v, \
             tc.tile_pool(name="p3x", bufs=2) as p3x, \
             tc.tile_pool(name="p3ps", bufs=4, space="PSUM") as p3ps:
            v_all = p3v.tile([128, NB * D], vdt)
            for dc in range(NB):
                nc.sync.dma_start(v_all[:, dc * D:(dc + 1) * D],
                                  Vw.ap()[dc * 128:(dc + 1) * 128, :])
            for ks in range(S // 512):
                xvs = p3x.tile([128, NB * 512], vdt)
                for dc in range(NB):
                    nc.sync.dma_start(xvs[:, dc * 512:(dc + 1) * 512],
                                      xvT.ap()[dc * 128:(dc + 1) * 128, ks * 512:(ks + 1) * 512])
                for ktl in range(4):
                    kc = ks * 4 + ktl
                    for dsl in range(2):
                        ps = p3ps.tile([128, 512], f32)
                        for dc in range(NB):
                            nc.tensor.matmul(
                                ps[:],
                                xvs[:, dc * 512 + ktl * 128: dc * 512 + (ktl + 1) * 128],
                                v_all[:, dc * D + dsl * 512: dc * D + (dsl + 1) * 512],
                                start=(dc == 0), stop=(dc == NB - 1))
                        nc.scalar.copy(vx[:, kc * D + dsl * 512: kc * D + (dsl + 1) * 512], ps[:])

        # ---- Attention, one slot (128 queries) at a time -------------------
        with tc.tile_pool(name="aq", bufs=2) as aq, \
             tc.tile_pool(name="am", bufs=1) as am, \
             tc.tile_pool(name="asc", bufs=2) as asc, \
             tc.tile_pool(name="apt", bufs=2) as apt, \
             tc.tile_pool(name="aou", bufs=1) as aou, \
             tc.tile_pool(name="ast", bufs=4) as ast, \
             tc.tile_pool(name="aps", bufs=3, space="PSUM") as aps, \
             tc.tile_pool(name="atp", bufs=2, space="PSUM") as atp, \
             tc.tile_pool(name="aop", bufs=2, space="PSUM") as aop:
            for i in range(NSLOT):
                nk = BETA[i]
                kw = nk * 128
                qxt_s = aq.tile([128, NB * 128], f32)
                for dc in range(NB):
                    nc.sync.dma_start(qxt_s[:, dc * 128:(dc + 1) * 128],
                                      qxt_d[dc * 128:(dc + 1) * 128, i * 128:(i + 1) * 128])
                mask_s = am.tile([128, S], f32)
                nc.sync.dma_start(mask_s[:, :kw], maskT.ap()[i * 128:(i + 1) * 128, :kw])

                scores = asc.tile([128, S], f32)
                off = 0
                while off < kw:
                    w = min(512, kw - off)
                    ps = aps.tile([128, 512], f32)
                    for dc in range(NB):
                        nc.tensor.matmul(
                            ps[:, :w],
                            qxt_s[:, dc * 128:(dc + 1) * 128],
                            kxt[:, dc * S + off: dc * S + off + w],
                            start=(dc == 0), stop=(dc == NB - 1))
                    nc.vector.tensor_add(scores[:, off:off + w], ps[:, :w], mask_s[:, off:off + w])
                    off += w

                negm = ast.tile([128, 1], f32)
                nc.vector.reduce_max(negm[:], scores[:, :kw], axis=AX, negate=True)
                probs = scores  # exp in place
                denom = ast.tile([128, 1], f32)
                nc.scalar.activation(probs[:, :kw], scores[:, :kw], Exp,
                                     bias=negm[:], scale=1.0, accum_out=denom[:])
                rec = ast.tile([128, 1], f32)
                nc.vector.reciprocal(rec[:], denom[:])
                nc.vector.tensor_scalar_mul(probs[:, :kw], probs[:, :kw], rec[:])
                nc.sync.dma_start(probs_o.ap()[i * 128:(i + 1) * 128, :kw], probs[:, :kw])

                pT = apt.tile([128, S], vdt)
                for kc in range(nk):
                    tps = atp.tile([128, 128], f32)
                    nc.tensor.transpose(tps[:], probs[:, kc * 128:(kc + 1) * 128], ident[:])
                    nc.vector.tensor_copy(pT[:, kc * 128:(kc + 1) * 128], tps[:])

                outsb = aou.tile([128, D], f32)
                for dsl in range(2):
                    ps = aop.tile([128, 512], f32)
                    for kc in range(nk):
                        nc.tensor.matmul(
                            ps[:],
                            pT[:, kc * 128:(kc + 1) * 128],
                            vx[:, kc * D + dsl * 512: kc * D + (dsl + 1) * 512],
                            start=(kc == 0), stop=(kc == nk - 1))
                    nc.scalar.copy(outsb[:, dsl * 512:(dsl + 1) * 512], ps[:])
                nc.sync.dma_start(out_o.ap()[i * 128:(i + 1) * 128, :], outsb[:])

    nc.compile()
    return nc


def _qtiles(par: int):
    """Global q-tile indices for parity `par`, descending causal need."""
    return [t for t in range(KT - 1, -1, -1) if t % 2 == par]


def kernel(x, Q, K, V, F):
    global LAST_RESULTS
    try:
        from antenv.axon_hooks import get_axon_ntff_profile_hook  # noqa: F401
    except Exception:
        # tracing would crash without the axon NTFF hook module
        os.environ["BASS_NEVER_TRACE"] = "1"
    from concourse.bass_utils import run_bass_kernel_spmd

    x = np.ascontiguousarray(x, dtype=np.float32)
    Q = np.ascontiguousarray(Q, dtype=np.float32)
    K = np.ascontiguousarray(K, dtype=np.float32)
    V = np.ascontiguousarray(V, dtype=np.float32)
    F = np.ascontiguousarray(F, dtype=np.float32)

    if np.array_equal(F, np.eye(D, dtype=np.float32)):
        xf = x
    else:
        xf = np.matmul(x, F)

    if "nc" not in _CACHE:
        _CACHE["nc"] = _build_nc()
    nc = _CACHE["nc"]

    Vr = _round_fp32r(V) if USE_FP32R else V

    # host-side per-core tensors
    xT_b = [np.ascontiguousarray(xf[b].T) for b in range(B)]
    xvT_b = [_round_fp32r(x[b].T) if USE_FP32R else np.ascontiguousarray(x[b].T)
             for b in range(B)]

    kpos = np.arange(S, dtype=np.int64)
    masks, qsels = {}, {}
    for par in (0, 1):
        tiles = _qtiles(par)
        qpos = np.concatenate([np.arange(t * 128, (t + 1) * 128, dtype=np.int64)
                               for t in tiles])
        masks[par] = np.where(kpos[None, :] <= qpos[:, None],
                              np.float32(0.0), np.float32(NEG)).astype(np.float32)
        qsels[par] = qpos

    in_maps = []
    for c in range(8):
        b, par = c // 2, c % 2
        in_maps.append({
            "xT": xT_b[b],
            "xqT": np.ascontiguousarray(xT_b[b][:, qsels[par]]),
            "xvT": xvT_b[b],
            "Qw": Q, "Kw": K, "Vw": Vr,
            "maskT": masks[par],
        })

    res = run_bass_kernel_spmd(nc, in_maps, list(range(8)))
    LAST_RESULTS = res

    probs = np.zeros((B, S, S), dtype=np.float32)
    out = np.zeros((B, S, D), dtype=np.float32)
    for c in range(8):
        b, par = c // 2, c % 2
        tiles = _qtiles(par)
        pc = res.results[c]["probs_o"]
        oc = res.results[c]["out_o"]
        for i, t in enumerate(tiles):
            probs[b, t * 128:(t + 1) * 128, :] = pc[i * 128:(i + 1) * 128, :]
            out[b, t * 128:(t + 1) * 128, :] = oc[i * 128:(i + 1) * 128, :]
    return (out, probs)


# revision 4
# speedup vs baseline: 1.1140x; 1.0190x over previous
"""Causal dot-product attention (B=4, S=2048, D=1024, single head) on 8 TRN2 cores.

Sharding: core c = (batch c//2, q-tile parity c%2).  Each core computes the
projections for its batch and attention for its 8 query tiles of 128 rows
(odd or even global q-tiles, descending causal key-need), so the per-slot
key bounds BETA are identical across cores (SPMD-uniform) and causality
skips 44% of the attention FLOPs.

Numerics: QxT and the score matmul run in true fp32.  KxT uses a 3-pass
fp32r hi/lo split (fp32-grade accuracy at the full-rate 1 cyc/row instead of
fp32's 4) with hi/lo pre-split on host.  The V path runs single-pass fp32r
(~5e-4 relative on `out`).  Scores are pre-scaled by 1/sqrt(d) in the QxT
projection copy; softmax max/exp/sum run on DVE/ACT with a fused row-sum.

Schedule: P1 QxT (spilled to DRAM) overlaps the prefetch of P2's first
slices (throttled mid-P1 so startup loads keep full HBM bandwidth);
P2 writes KxT to SBUF direct; the Vx pool is deferred past P2 to keep SBUF
under the cap; attention streams one 128-query slot at a time.
"""

import math
import os
from contextlib import ExitStack

import numpy as np

B, S, D = 4, 2048, 1024
NB = D // 128          # 8 contraction chunks
KT = S // 128          # 16 key tiles
NSLOT = 8              # query tiles per core
QL = NSLOT * 128       # local query rows per core (1024)
SCALE = 1.0 / math.sqrt(D)
NEG = -1.0e30

CAUSAL = os.environ.get("KB_CAUSAL", "1") == "1"
USE_FP32R = os.environ.get("KB_FP32R", "1") == "1"
BETA = [16, 14, 12, 10, 8, 6, 4, 2] if CAUSAL else [16] * 8   # k-tiles per slot

_CACHE = {}
LAST_RESULTS = None


def _round_fp32r(a: np.ndarray) -> np.ndarray:
    """Round-to-nearest-even to 11 mantissa bits (the fp32r grid)."""
    b = np.ascontiguousarray(a, dtype=np.float32).view(np.uint32)
    r = (b + np.uint32(0x7FF) + ((b >> np.uint32(12)) & np.uint32(1))) & np.uint32(0xFFFFF000)
    return r.view(np.float32)


def _split_fp32r(a):
    """a ~= hi + lo with both on the fp32r grid (hi 11-bit, lo the residual)."""
    hi = _round_fp32r(a)
    lo = _round_fp32r(np.asarray(a, dtype=np.float32) - hi)
    return hi, lo


def _build_nc():
    import concourse.tile as tile
    from concourse import bacc, mybir
    from concourse.masks import make_identity

    f32 = mybir.dt.float32
    f32r = mybir.dt.float32r
    vdt = f32r if USE_FP32R else f32

    nc = bacc.Bacc("TRN2", target_bir_lowering=False, debug=False)

    xThh = nc.dram_tensor("xThh", [D, S], f32r, kind="ExternalInput")  # hi(xf[b].T)
    xThl = nc.dram_tensor("xThl", [D, S], f32r, kind="ExternalInput")  # lo(xf[b].T)
    xqT = nc.dram_tensor("xqT", [D, QL], f32, kind="ExternalInput")    # query cols of xf[b].T
    xvT = nc.dram_tensor("xvT", [D, S], vdt, kind="ExternalInput")     # x[b].T (rounded)
    Qw = nc.dram_tensor("Qw", [D, D], f32, kind="ExternalInput")
    Kh = nc.dram_tensor("Kh", [D, D], f32r, kind="ExternalInput")
    Kl = nc.dram_tensor("Kl", [D, D], f32r, kind="ExternalInput")
    Vw = nc.dram_tensor("Vw", [D, D], vdt, kind="ExternalInput")       # rounded
    maskT = nc.dram_tensor("maskT", [QL, S], f32, kind="ExternalInput")
    probs_o = nc.dram_tensor("probs_o", [QL, S], f32, kind="ExternalOutput")
    out_o = nc.dram_tensor("out_o", [QL, D], f32, kind="ExternalOutput")

    Exp = mybir.ActivationFunctionType.Exp
    AX = mybir.AxisListType.X

    with tile.TileContext(nc) as tc, ExitStack() as ctx:
        const_pool = ctx.enter_context(tc.tile_pool(name="const", bufs=1))
        ident = const_pool.tile([128, 128], f32)
        make_identity(nc, ident[:])

        kxt_pool = ctx.enter_context(tc.tile_pool(name="kxt", bufs=1))
        kxt = kxt_pool.tile([128, NB * S], f32)          # [d' in chunk, dc*S + k]
        qxtd_pool = ctx.enter_context(tc.tile_pool(name="qxtd", bufs=1, space="DRAM"))
        qxt_d = qxtd_pool.tile([D, QL], f32)

        # ---- P2 pools first so their initial loads run during P1 ----------
        with tc.tile_pool(name="p2k", bufs=2) as p2k, \
             tc.tile_pool(name="p2x", bufs=2) as p2x, \
             tc.tile_pool(name="p2xl", bufs=2) as p2xl:
            xs_tiles = []
            for ks in range(S // 512):
                xsh = p2x.tile([128, NB * 512], f32r, tag="xsh")
                xsl = p2xl.tile([128, NB * 512], f32r, tag="xsl")
                xs_tiles.append((xsh, xsl))
            kc_tiles = []
            for _j in range(2):
                kct = p2k.tile([128, 2 * NB * 128], f32r, tag="kc")
                kc_tiles.append(kct)

            # ---- P1: QxT = (xq @ Q).T * SCALE  (fp32) -> DRAM spill --------
            with tc.tile_pool(name="p1sb", bufs=1) as p1sb, \
                 tc.tile_pool(name="p1q", bufs=2) as p1q, \
                 tc.tile_pool(name="p1st", bufs=1) as p1st, \
                 tc.tile_pool(name="p1ps", bufs=4, space="PSUM") as p1ps:
                xq_all = p1sb.tile([128, NB * QL], f32)
                for dc in range(NB):
                    nc.sync.dma_start(xq_all[:, dc * QL:(dc + 1) * QL],
                                      xqT.ap()[dc * 128:(dc + 1) * 128, :])
                for dt_ in range(NB):
                    qcol = p1q.tile([128, NB * 128], f32, tag="qc")
                    nc.sync.dma_start(
                        qcol[:].rearrange("p (c m) -> p c m", c=NB),
                        Qw.ap()[:, dt_ * 128:(dt_ + 1) * 128].rearrange("(c p) m -> p c m", p=128))
                    if dt_ == 3:
                        # prefetch P2's first x-slices + first K columns now --
                        # the qcol slot wait above throttles these behind the
                        # startup-critical loads
                        xsh0, xsl0 = xs_tiles[0]
                        nc.sync.dma_start(
                            xsh0[:].rearrange("p (c m) -> p c m", c=NB),
                            xThh.ap()[:, 0:512].rearrange("(c p) m -> p c m", p=128))
                        nc.sync.dma_start(
                            xsl0[:].rearrange("p (c m) -> p c m", c=NB),
                            xThl.ap()[:, 0:512].rearrange("(c p) m -> p c m", p=128))
                        for j in range(2):
                            nc.sync.dma_start(
                                kc_tiles[j][:, :NB * 128].rearrange("p (c m) -> p c m", c=NB),
                                Kh.ap()[:, j * 128:(j + 1) * 128].rearrange("(c p) m -> p c m", p=128))
                            nc.sync.dma_start(
                                kc_tiles[j][:, NB * 128:].rearrange("p (c m) -> p c m", c=NB),
                                Kl.ap()[:, j * 128:(j + 1) * 128].rearrange("(c p) m -> p c m", p=128))
                    stage = p1st.tile([128, QL], f32)
                    for qs in range(QL // 512):
                        ps = p1ps.tile([128, 512], f32)
                        for dc in range(NB):
                            nc.tensor.matmul(
                                ps[:],
                                qcol[:, dc * 128:(dc + 1) * 128],
                                xq_all[:, dc * QL + qs * 512: dc * QL + (qs + 1) * 512],
                                start=(dc == 0), stop=(dc == NB - 1))
                        nc.scalar.mul(stage[:, qs * 512:(qs + 1) * 512], ps[:], SCALE)
                    nc.gpsimd.dma_start(qxt_d[dt_ * 128:(dt_ + 1) * 128, :], stage[:])

            # ---- P2: KxT = (x @ K).T via 3-pass fp32r hi/lo split ----------
            with tc.tile_pool(name="p2ps", bufs=4, space="PSUM") as p2ps:
                kci = 2
                for ks in range(S // 512):
                    xsh, xsl = xs_tiles[ks]
                    if ks > 0:
                        nc.sync.dma_start(
                            xsh[:].rearrange("p (c m) -> p c m", c=NB),
                            xThh.ap()[:, ks * 512:(ks + 1) * 512].rearrange("(c p) m -> p c m", p=128))
                        nc.sync.dma_start(
                            xsl[:].rearrange("p (c m) -> p c m", c=NB),
                            xThl.ap()[:, ks * 512:(ks + 1) * 512].rearrange("(c p) m -> p c m", p=128))
                    for dt_ in range(NB):
                        if ks == 0 and dt_ < 2:
                            kc_hl = kc_tiles[dt_]
                        else:
                            kc_hl = p2k.tile([128, 2 * NB * 128], f32r, tag="kc")
                            nc.sync.dma_start(
                                kc_hl[:, :NB * 128].rearrange("p (c m) -> p c m", c=NB),
                                Kh.ap()[:, dt_ * 128:(dt_ + 1) * 128].rearrange("(c p) m -> p c m", p=128))
                            nc.sync.dma_start(
                                kc_hl[:, NB * 128:].rearrange("p (c m) -> p c m", c=NB),
                                Kl.ap()[:, dt_ * 128:(dt_ + 1) * 128].rearrange("(c p) m -> p c m", p=128))
                        kcol_h = kc_hl[:, :NB * 128]
                        kcol_l = kc_hl[:, NB * 128:]
                        ps = p2ps.tile([128, 512], f32)
                        i_mm, nmm = 0, 3 * NB
                        for wt, xs in ((kcol_h, xsh), (kcol_h, xsl), (kcol_l, xsh)):
                            for dc in range(NB):
                                nc.tensor.matmul(
                                    ps[:],
                                    wt[:, dc * 128:(dc + 1) * 128],
                                    xs[:, dc * 512:(dc + 1) * 512],
                                    start=(i_mm == 0), stop=(i_mm == nmm - 1))
                                i_mm += 1
                        nc.scalar.copy(kxt[:, dt_ * S + ks * 512: dt_ * S + (ks + 1) * 512], ps[:])

        vx_pool = ctx.enter_context(tc.tile_pool(name="vx", bufs=1))
        vx = vx_pool.tile([128, KT * D], vdt)            # [k in chunk, kc*D + d']

        # ---- P3: Vx = x @ V  (fp32r, resident) -----------------------------
        with tc.tile_pool(name="p3v", bufs=1) as p3v, \
             tc.tile_pool(name="p3x", bufs=2) as p3x, \
             tc.tile_pool(name="p3ps", bufs=4, space="PSUM") as p3ps:
            v_all = p3v.tile([128, NB * D], vdt)
            for dc in range(NB):
                nc.sync.dma_start(v_all[:, dc * D:(dc + 1) * D],
                                  Vw.ap()[dc * 128:(dc + 1) * 128, :])
            for ks in range(S // 512):
                xvs = p3x.tile([128, NB * 512], vdt)
                for dc in range(NB):
                    nc.sync.dma_start(xvs[:, dc * 512:(dc + 1) * 512],
                                      xvT.ap()[dc * 128:(dc + 1) * 128, ks * 512:(ks + 1) * 512])
                for ktl in range(4):
                    kc = ks * 4 + ktl
                    for dsl in range(2):
                        ps = p3ps.tile([128, 512], f32)
                        for dc in range(NB):
                            nc.tensor.matmul(
                                ps[:],
                                xvs[:, dc * 512 + ktl * 128: dc * 512 + (ktl + 1) * 128],
                                v_all[:, dc * D + dsl * 512: dc * D + (dsl + 1) * 512],
                                start=(dc == 0), stop=(dc == NB - 1))
                        nc.scalar.copy(vx[:, kc * D + dsl * 512: kc * D + (dsl + 1) * 512], ps[:])

        # ---- Attention, one slot (128 queries) at a time -------------------
        with tc.tile_pool(name="aq", bufs=2) as aq, \
             tc.tile_pool(name="am", bufs=1) as am, \
             tc.tile_pool(name="asc", bufs=2) as asc, \
             tc.tile_pool(name="apt", bufs=2) as apt, \
             tc.tile_pool(name="aou", bufs=1) as aou, \
             tc.tile_pool(name="ast", bufs=4) as ast, \
             tc.tile_pool(name="aps", bufs=3, space="PSUM") as aps, \
             tc.tile_pool(name="atp", bufs=2, space="PSUM") as atp, \
             tc.tile_pool(name="aop", bufs=2, space="PSUM") as aop:
            for i in range(NSLOT):
                nk = BETA[i]
                kw = nk * 128
                qxt_s = aq.tile([128, NB * 128], f32)
                for dc in range(NB):
                    nc.sync.dma_start(qxt_s[:, dc * 128:(dc + 1) * 128],
                                      qxt_d[dc * 128:(dc + 1) * 128, i * 128:(i + 1) * 128])
                mask_s = am.tile([128, S], f32)
                nc.sync.dma_start(mask_s[:, :kw], maskT.ap()[i * 128:(i + 1) * 128, :kw])

                scores = asc.tile([128, S], f32)
                off = 0
                while off < kw:
                    w = min(512, kw - off)
                    ps = aps.tile([128, 512], f32)
                    for dc in range(NB):
                        nc.tensor.matmul(
                            ps[:, :w],
                            qxt_s[:, dc * 128:(dc + 1) * 128],
                            kxt[:, dc * S + off: dc * S + off + w],
                            start=(dc == 0), stop=(dc == NB - 1))
                    nc.vector.tensor_add(scores[:, off:off + w], ps[:, :w], mask_s[:, off:off + w])
                    off += w

                negm = ast.tile([128, 1], f32)
                nc.vector.reduce_max(negm[:], scores[:, :kw], axis=AX, negate=True)
                probs = scores  # exp in place
                denom = ast.tile([128, 1], f32)
                nc.scalar.activation(probs[:, :kw], scores[:, :kw], Exp,
                                     bias=negm[:], scale=1.0, accum_out=denom[:])
                rec = ast.tile([128, 1], f32)
                nc.vector.reciprocal(rec[:], denom[:])
                nc.vector.tensor_scalar_mul(probs[:, :kw], probs[:, :kw], rec[:])
                nc.sync.dma_start(probs_o.ap()[i * 128:(i + 1) * 128, :kw], probs[:, :kw])

                pT = apt.tile([128, S], vdt)
                for kc in range(nk):
                    tps = atp.tile([128, 128], f32)
                    nc.tensor.transpose(tps[:], probs[:, kc * 128:(kc + 1) * 128], ident[:])
                    nc.vector.tensor_copy(pT[:, kc * 128:(kc + 1) * 128], tps[:])

                outsb = aou.tile([128, D], f32)
                for dsl in range(2):
                    ps = aop.tile([128, 512], f32)
                    for kc in range(nk):
                        nc.tensor.matmul(
                            ps[:],
                            pT[:, kc * 128:(kc + 1) * 128],
                            vx[:, kc * D + dsl * 512: kc * D + (dsl + 1) * 512],
                            start=(kc == 0), stop=(kc == nk - 1))
                    nc.scalar.copy(outsb[:, dsl * 512:(dsl + 1) * 512], ps[:])
                nc.sync.dma_start(out_o.ap()[i * 128:(i + 1) * 128, :], outsb[:])

    nc.compile()
    return nc


def _qtiles(par: int):
    """Global q-tile indices for parity `par`, descending causal need."""
    return [t for t in range(KT - 1, -1, -1) if t % 2 == par]


def kernel(x, Q, K, V, F):
    global LAST_RESULTS
    try:
        from antenv.axon_hooks import get_axon_ntff_profile_hook  # noqa: F401
    except Exception:
        # tracing would crash without the axon NTFF hook module
        os.environ["BASS_NEVER_TRACE"] = "1"
    from concourse.bass_utils import run_bass_kernel_spmd

    x = np.ascontiguousarray(x, dtype=np.float32)
    Q = np.ascontiguousarray(Q, dtype=np.float32)
    K = np.ascontiguousarray(K, dtype=np.float32)
    V = np.ascontiguousarray(V, dtype=np.float32)
    F = np.ascontiguousarray(F, dtype=np.float32)

    if np.array_equal(F, np.eye(D, dtype=np.float32)):
        xf = x
    else:
        xf = np.matmul(x, F)

    if "nc" not in _CACHE:
        _CACHE["nc"] = _build_nc()
    nc = _CACHE["nc"]

    Vr = _round_fp32r(V) if USE_FP32R else V
    Kh_, Kl_ = _split_fp32r(K)

    # host-side per-core tensors
    xT_b = [np.ascontiguousarray(xf[b].T) for b in range(B)]
    xTs_b = [_split_fp32r(xT_b[b]) for b in range(B)]
    xvT_b = [_round_fp32r(x[b].T) if USE_FP32R else np.ascontiguousarray(x[b].T)
             for b in range(B)]

    kpos = np.arange(S, dtype=np.int64)
    masks, qsels = {}, {}
    for par in (0, 1):
        tiles = _qtiles(par)
        qpos = np.concatenate([np.arange(t * 128, (t + 1) * 128, dtype=np.int64)
                               for t in tiles])
        masks[par] = np.where(kpos[None, :] <= qpos[:, None],
                              np.float32(0.0), np.float32(NEG)).astype(np.float32)
        qsels[par] = qpos

    in_maps = []
    for c in range(8):
        b, par = c // 2, c % 2
        in_maps.append({
            "xThh": xTs_b[b][0], "xThl": xTs_b[b][1],
            "xqT": np.ascontiguousarray(xT_b[b][:, qsels[par]]),
            "xvT": xvT_b[b],
            "Qw": Q, "Kh": Kh_, "Kl": Kl_, "Vw": Vr,
            "maskT": masks[par],
        })

    res = run_bass_kernel_spmd(nc, in_maps, list(range(8)))
    LAST_RESULTS = res

    probs = np.zeros((B, S, S), dtype=np.float32)
    out = np.zeros((B, S, D), dtype=np.float32)
    for c in range(8):
        b, par = c // 2, c % 2
        tiles = _qtiles(par)
        pc = res.results[c]["probs_o"]
        oc = res.results[c]["out_o"]
        for i, t in enumerate(tiles):
            probs[b, t * 128:(t + 1) * 128, :] = pc[i * 128:(i + 1) * 128, :]
            out[b, t * 128:(t + 1) * 128, :] = oc[i * 128:(i + 1) * 128, :]
    return (out, probs)


# revision 5
# speedup vs baseline: 1.1159x; 1.0017x over previous
"""Causal dot-product attention (B=4, S=2048, D=1024, single head) on 8 TRN2 cores.

HW exec time ~570 us/core; scale-relative max-abs error ~3.8e-4 on probs+out.

Sharding: core c = (batch c//2, q-tile parity c%2).  Each core computes the
projections for its batch and attention for its 8 query tiles of 128 rows
(odd or even global q-tiles, descending causal key-need), so the per-slot
key bounds BETA are identical across cores (SPMD-uniform) and causality
skips 44% of the attention FLOPs.  Host gathers query columns per core and
scatters result rows back; the causal mask is an additive per-core input.

Numerics: QxT and the score matmul run in true fp32 (hardware-verified
fp32-accurate; scores have std ~1700 so score error must stay << 1).
KxT uses a 3-pass fp32r hi/lo split (K.T@x ~= Khi@xhi + Khi@xlo + Klo@xhi,
host pre-split; fp32-grade accuracy at 1 cyc/row instead of fp32's 4).
The V path is single-pass fp32r (11-bit mantissa, ~5e-4 relative on `out`).
1/sqrt(d) is folded into the QxT projection copy; softmax runs max/exp with
a fused row-sum (accum_out) and normalizes on-device.

Schedule: P1 QxT (fp32, spilled to DRAM) runs first; P2's pools are created
outside P1 so P2's first slices+K columns prefetch mid-P1 (throttled behind a
buffer-slot wait so startup loads keep full HBM bandwidth); P2 writes KxT to
SBUF direct via 3-pass psum groups; the Vx pool is deferred past P2 to fit
SBUF; attention streams one 128-query slot at a time (scores -> masked
softmax -> probs DMA -> PE-transpose -> probs@V -> out DMA).
"""

import math
import os
from contextlib import ExitStack

import numpy as np

B, S, D = 4, 2048, 1024
NB = D // 128          # 8 contraction chunks
KT = S // 128          # 16 key tiles
NSLOT = 8              # query tiles per core
QL = NSLOT * 128       # local query rows per core (1024)
SCALE = 1.0 / math.sqrt(D)
NEG = -1.0e30

CAUSAL = os.environ.get("KB_CAUSAL", "1") == "1"
USE_FP32R = os.environ.get("KB_FP32R", "1") == "1"
BETA = [16, 14, 12, 10, 8, 6, 4, 2] if CAUSAL else [16] * 8   # k-tiles per slot

_CACHE = {}
LAST_RESULTS = None


def _round_fp32r(a: np.ndarray) -> np.ndarray:
    """Round-to-nearest-even to 11 mantissa bits (the fp32r grid)."""
    b = np.ascontiguousarray(a, dtype=np.float32).view(np.uint32)
    r = (b + np.uint32(0x7FF) + ((b >> np.uint32(12)) & np.uint32(1))) & np.uint32(0xFFFFF000)
    return r.view(np.float32)


def _split_fp32r(a):
    """a ~= hi + lo with both on the fp32r grid (hi 11-bit, lo the residual)."""
    hi = _round_fp32r(a)
    lo = _round_fp32r(np.asarray(a, dtype=np.float32) - hi)
    return hi, lo


def _build_nc():
    import concourse.tile as tile
    from concourse import bacc, mybir
    from concourse.masks import make_identity

    f32 = mybir.dt.float32
    f32r = mybir.dt.float32r
    vdt = f32r if USE_FP32R else f32

    nc = bacc.Bacc("TRN2", target_bir_lowering=False, debug=False)

    xThh = nc.dram_tensor("xThh", [D, S], f32r, kind="ExternalInput")  # hi(xf[b].T)
    xThl = nc.dram_tensor("xThl", [D, S], f32r, kind="ExternalInput")  # lo(xf[b].T)
    xqT = nc.dram_tensor("xqT", [D, QL], f32, kind="ExternalInput")    # query cols of xf[b].T
    xvT = nc.dram_tensor("xvT", [D, S], vdt, kind="ExternalInput")     # x[b].T (rounded)
    Qw = nc.dram_tensor("Qw", [D, D], f32, kind="ExternalInput")
    Kh = nc.dram_tensor("Kh", [D, D], f32r, kind="ExternalInput")
    Kl = nc.dram_tensor("Kl", [D, D], f32r, kind="ExternalInput")
    Vw = nc.dram_tensor("Vw", [D, D], vdt, kind="ExternalInput")       # rounded
    maskT = nc.dram_tensor("maskT", [QL, S], f32, kind="ExternalInput")
    probs_o = nc.dram_tensor("probs_o", [QL, S], f32, kind="ExternalOutput")
    out_o = nc.dram_tensor("out_o", [QL, D], f32, kind="ExternalOutput")

    Exp = mybir.ActivationFunctionType.Exp
    AX = mybir.AxisListType.X

    with tile.TileContext(nc) as tc, ExitStack() as ctx:
        const_pool = ctx.enter_context(tc.tile_pool(name="const", bufs=1))
        ident = const_pool.tile([128, 128], f32)
        make_identity(nc, ident[:])

        kxt_pool = ctx.enter_context(tc.tile_pool(name="kxt", bufs=1))
        kxt = kxt_pool.tile([128, NB * S], f32)          # [d' in chunk, dc*S + k]
        qxtd_pool = ctx.enter_context(tc.tile_pool(name="qxtd", bufs=1, space="DRAM"))
        qxt_d = qxtd_pool.tile([D, QL], f32)

        # ---- P2 pools first so their initial loads run during P1 ----------
        with tc.tile_pool(name="p2k", bufs=2) as p2k, \
             tc.tile_pool(name="p2x", bufs=2) as p2x, \
             tc.tile_pool(name="p2xl", bufs=2) as p2xl:
            xs_tiles = []
            for ks in range(S // 512):
                xsh = p2x.tile([128, NB * 512], f32r, tag="xsh")
                xsl = p2xl.tile([128, NB * 512], f32r, tag="xsl")
                xs_tiles.append((xsh, xsl))
            kc_tiles = []
            for _j in range(2):
                kct = p2k.tile([128, 2 * NB * 128], f32r, tag="kc")
                kc_tiles.append(kct)

            # ---- P1: QxT = (xq @ Q).T * SCALE  (fp32) -> DRAM spill --------
            with tc.tile_pool(name="p1sb", bufs=1) as p1sb, \
                 tc.tile_pool(name="p1q", bufs=2) as p1q, \
                 tc.tile_pool(name="p1st", bufs=1) as p1st, \
                 tc.tile_pool(name="p1ps", bufs=4, space="PSUM") as p1ps:
                xq_all = p1sb.tile([128, NB * QL], f32)
                for qs in range(QL // 512):
                    for dc in range(NB):
                        nc.sync.dma_start(
                            xq_all[:, dc * QL + qs * 512: dc * QL + (qs + 1) * 512],
                            xqT.ap()[dc * 128:(dc + 1) * 128, qs * 512:(qs + 1) * 512])
                for dt_ in range(NB):
                    qcol = p1q.tile([128, NB * 128], f32, tag="qc")
                    nc.sync.dma_start(
                        qcol[:].rearrange("p (c m) -> p c m", c=NB),
                        Qw.ap()[:, dt_ * 128:(dt_ + 1) * 128].rearrange("(c p) m -> p c m", p=128))
                    if dt_ == 3:
                        # prefetch P2's first x-slices + first K columns now --
                        # the qcol slot wait above throttles these behind the
                        # startup-critical loads
                        xsh0, xsl0 = xs_tiles[0]
                        nc.sync.dma_start(
                            xsh0[:].rearrange("p (c m) -> p c m", c=NB),
                            xThh.ap()[:, 0:512].rearrange("(c p) m -> p c m", p=128))
                        nc.sync.dma_start(
                            xsl0[:].rearrange("p (c m) -> p c m", c=NB),
                            xThl.ap()[:, 0:512].rearrange("(c p) m -> p c m", p=128))
                        for j in range(2):
                            nc.sync.dma_start(
                                kc_tiles[j][:, :NB * 128].rearrange("p (c m) -> p c m", c=NB),
                                Kh.ap()[:, j * 128:(j + 1) * 128].rearrange("(c p) m -> p c m", p=128))
                            nc.sync.dma_start(
                                kc_tiles[j][:, NB * 128:].rearrange("p (c m) -> p c m", c=NB),
                                Kl.ap()[:, j * 128:(j + 1) * 128].rearrange("(c p) m -> p c m", p=128))
                    stage = p1st.tile([128, QL], f32)
                    for qs in range(QL // 512):
                        ps = p1ps.tile([128, 512], f32)
                        for dc in range(NB):
                            nc.tensor.matmul(
                                ps[:],
                                qcol[:, dc * 128:(dc + 1) * 128],
                                xq_all[:, dc * QL + qs * 512: dc * QL + (qs + 1) * 512],
                                start=(dc == 0), stop=(dc == NB - 1))
                        nc.scalar.mul(stage[:, qs * 512:(qs + 1) * 512], ps[:], SCALE)
                    nc.gpsimd.dma_start(qxt_d[dt_ * 128:(dt_ + 1) * 128, :], stage[:])

            # ---- P2: KxT = (x @ K).T via 3-pass fp32r hi/lo split ----------
            with tc.tile_pool(name="p2ps", bufs=4, space="PSUM") as p2ps:
                kci = 2
                for ks in range(S // 512):
                    xsh, xsl = xs_tiles[ks]
                    if ks > 0:
                        for g in range(2):
                            nc.sync.dma_start(
                                xsh[:].rearrange("p (c m) -> p c m", c=NB)[:, g * 4:(g + 1) * 4, :],
                                xThh.ap()[g * 512:(g + 1) * 512, ks * 512:(ks + 1) * 512]
                                .rearrange("(c p) m -> p c m", p=128))
                            nc.sync.dma_start(
                                xsl[:].rearrange("p (c m) -> p c m", c=NB)[:, g * 4:(g + 1) * 4, :],
                                xThl.ap()[g * 512:(g + 1) * 512, ks * 512:(ks + 1) * 512]
                                .rearrange("(c p) m -> p c m", p=128))
                    for dt_ in range(NB):
                        if ks == 0 and dt_ < 2:
                            kc_hl = kc_tiles[dt_]
                        else:
                            kc_hl = p2k.tile([128, 2 * NB * 128], f32r, tag="kc")
                            nc.sync.dma_start(
                                kc_hl[:, :NB * 128].rearrange("p (c m) -> p c m", c=NB),
                                Kh.ap()[:, dt_ * 128:(dt_ + 1) * 128].rearrange("(c p) m -> p c m", p=128))
                            nc.sync.dma_start(
                                kc_hl[:, NB * 128:].rearrange("p (c m) -> p c m", c=NB),
                                Kl.ap()[:, dt_ * 128:(dt_ + 1) * 128].rearrange("(c p) m -> p c m", p=128))
                        kcol_h = kc_hl[:, :NB * 128]
                        kcol_l = kc_hl[:, NB * 128:]
                        ps = p2ps.tile([128, 512], f32)
                        i_mm, nmm = 0, 3 * NB
                        for wt, xs in ((kcol_h, xsh), (kcol_h, xsl), (kcol_l, xsh)):
                            for dc in range(NB):
                                nc.tensor.matmul(
                                    ps[:],
                                    wt[:, dc * 128:(dc + 1) * 128],
                                    xs[:, dc * 512:(dc + 1) * 512],
                                    start=(i_mm == 0), stop=(i_mm == nmm - 1))
                                i_mm += 1
                        nc.scalar.copy(kxt[:, dt_ * S + ks * 512: dt_ * S + (ks + 1) * 512], ps[:])

        vx_pool = ctx.enter_context(tc.tile_pool(name="vx", bufs=1))
        vx = vx_pool.tile([128, KT * D], vdt)            # [k in chunk, kc*D + d']

        # ---- P3: Vx = x @ V  (fp32r, resident) -----------------------------
        with tc.tile_pool(name="p3v", bufs=1) as p3v, \
             tc.tile_pool(name="p3x", bufs=2) as p3x, \
             tc.tile_pool(name="p3ps", bufs=4, space="PSUM") as p3ps:
            v_all = p3v.tile([128, NB * D], vdt)
            for dc in range(NB):
                nc.sync.dma_start(v_all[:, dc * D:(dc + 1) * D],
                                  Vw.ap()[dc * 128:(dc + 1) * 128, :])
            for ks in range(S // 512):
                xvs = p3x.tile([128, NB * 512], vdt)
                for dc in range(NB):
                    nc.sync.dma_start(xvs[:, dc * 512:(dc + 1) * 512],
                                      xvT.ap()[dc * 128:(dc + 1) * 128, ks * 512:(ks + 1) * 512])
                for ktl in range(4):
                    kc = ks * 4 + ktl
                    for dsl in range(2):
                        ps = p3ps.tile([128, 512], f32)
                        for dc in range(NB):
                            nc.tensor.matmul(
                                ps[:],
                                xvs[:, dc * 512 + ktl * 128: dc * 512 + (ktl + 1) * 128],
                                v_all[:, dc * D + dsl * 512: dc * D + (dsl + 1) * 512],
                                start=(dc == 0), stop=(dc == NB - 1))
                        nc.scalar.copy(vx[:, kc * D + dsl * 512: kc * D + (dsl + 1) * 512], ps[:])

        # ---- Attention, one slot (128 queries) at a time -------------------
        with tc.tile_pool(name="aq", bufs=2) as aq, \
             tc.tile_pool(name="am", bufs=1) as am, \
             tc.tile_pool(name="asc", bufs=2) as asc, \
             tc.tile_pool(name="apt", bufs=2) as apt, \
             tc.tile_pool(name="aou", bufs=1) as aou, \
             tc.tile_pool(name="ast", bufs=4) as ast, \
             tc.tile_pool(name="aps", bufs=3, space="PSUM") as aps, \
             tc.tile_pool(name="atp", bufs=2, space="PSUM") as atp, \
             tc.tile_pool(name="aop", bufs=2, space="PSUM") as aop:
            for i in range(NSLOT):
                nk = BETA[i]
                kw = nk * 128
                qxt_s = aq.tile([128, NB * 128], f32)
                for dc in range(NB):
                    nc.sync.dma_start(qxt_s[:, dc * 128:(dc + 1) * 128],
                                      qxt_d[dc * 128:(dc + 1) * 128, i * 128:(i + 1) * 128])
                mask_s = am.tile([128, S], f32)
                nc.sync.dma_start(mask_s[:, :kw], maskT.ap()[i * 128:(i + 1) * 128, :kw])

                scores = asc.tile([128, S], f32)
                off = 0
                while off < kw:
                    w = min(512, kw - off)
                    ps = aps.tile([128, 512], f32)
                    for dc in range(NB):
                        nc.tensor.matmul(
                            ps[:, :w],
                            qxt_s[:, dc * 128:(dc + 1) * 128],
                            kxt[:, dc * S + off: dc * S + off + w],
                            start=(dc == 0), stop=(dc == NB - 1))
                    nc.vector.tensor_add(scores[:, off:off + w], ps[:, :w], mask_s[:, off:off + w])
                    off += w

                negm = ast.tile([128, 1], f32)
                nc.vector.reduce_max(negm[:], scores[:, :kw], axis=AX, negate=True)
                probs = scores  # exp in place
                denom = ast.tile([128, 1], f32)
                nc.scalar.activation(probs[:, :kw], scores[:, :kw], Exp,
                                     bias=negm[:], scale=1.0, accum_out=denom[:])
                rec = ast.tile([128, 1], f32)
                nc.vector.reciprocal(rec[:], denom[:])
                nc.vector.tensor_scalar_mul(probs[:, :kw], probs[:, :kw], rec[:])
                nc.sync.dma_start(probs_o.ap()[i * 128:(i + 1) * 128, :kw], probs[:, :kw])

                pT = apt.tile([128, S], vdt)
                for kc in range(nk):
                    tps = atp.tile([128, 128], f32)
                    nc.tensor.transpose(tps[:], probs[:, kc * 128:(kc + 1) * 128], ident[:])
                    nc.vector.tensor_copy(pT[:, kc * 128:(kc + 1) * 128], tps[:])

                outsb = aou.tile([128, D], f32)
                for dsl in range(2):
                    ps = aop.tile([128, 512], f32)
                    for kc in range(nk):
                        nc.tensor.matmul(
                            ps[:],
                            pT[:, kc * 128:(kc + 1) * 128],
                            vx[:, kc * D + dsl * 512: kc * D + (dsl + 1) * 512],
                            start=(kc == 0), stop=(kc == nk - 1))
                    nc.scalar.copy(outsb[:, dsl * 512:(dsl + 1) * 512], ps[:])
                nc.sync.dma_start(out_o.ap()[i * 128:(i + 1) * 128, :], outsb[:])

    nc.compile()
    return nc


def _qtiles(par: int):
    """Global q-tile indices for parity `par`, descending causal need."""
    return [t for t in range(KT - 1, -1, -1) if t % 2 == par]


def kernel(x, Q, K, V, F):
    global LAST_RESULTS
    try:
        from antenv.axon_hooks import get_axon_ntff_profile_hook  # noqa: F401
    except Exception:
        # tracing would crash without the axon NTFF hook module
        os.environ["BASS_NEVER_TRACE"] = "1"
    from concourse.bass_utils import run_bass_kernel_spmd

    x = np.ascontiguousarray(x, dtype=np.float32)
    Q = np.ascontiguousarray(Q, dtype=np.float32)
    K = np.ascontiguousarray(K, dtype=np.float32)
    V = np.ascontiguousarray(V, dtype=np.float32)
    F = np.ascontiguousarray(F, dtype=np.float32)

    if np.array_equal(F, np.eye(D, dtype=np.float32)):
        xf = x
    else:
        xf = np.matmul(x, F)

    if "nc" not in _CACHE:
        _CACHE["nc"] = _build_nc()
    nc = _CACHE["nc"]

    Vr = _round_fp32r(V) if USE_FP32R else V
    Kh_, Kl_ = _split_fp32r(K)

    # host-side per-core tensors
    xT_b = [np.ascontiguousarray(xf[b].T) for b in range(B)]
    xTs_b = [_split_fp32r(xT_b[b]) for b in range(B)]
    xvT_b = [_round_fp32r(x[b].T) if USE_FP32R else np.ascontiguousarray(x[b].T)
             for b in range(B)]

    kpos = np.arange(S, dtype=np.int64)
    masks, qsels = {}, {}
    for par in (0, 1):
        tiles = _qtiles(par)
        qpos = np.concatenate([np.arange(t * 128, (t + 1) * 128, dtype=np.int64)
                               for t in tiles])
        masks[par] = np.where(kpos[None, :] <= qpos[:, None],
                              np.float32(0.0), np.float32(NEG)).astype(np.float32)
        qsels[par] = qpos

    in_maps = []
    for c in range(8):
        b, par = c // 2, c % 2
        in_maps.append({
            "xThh": xTs_b[b][0], "xThl": xTs_b[b][1],
            "xqT": np.ascontiguousarray(xT_b[b][:, qsels[par]]),
            "xvT": xvT_b[b],
            "Qw": Q, "Kh": Kh_, "Kl": Kl_, "Vw": Vr,
            "maskT": masks[par],
        })

    res = run_bass_kernel_spmd(nc, in_maps, list(range(8)))
    LAST_RESULTS = res

    probs = np.zeros((B, S, S), dtype=np.float32)
    out = np.zeros((B, S, D), dtype=np.float32)
    for c in range(8):
        b, par = c // 2, c % 2
        tiles = _qtiles(par)
        pc = res.results[c]["probs_o"]
        oc = res.results[c]["out_o"]
        for i, t in enumerate(tiles):
            probs[b, t * 128:(t + 1) * 128, :] = pc[i * 128:(i + 1) * 128, :]
            out[b, t * 128:(t + 1) * 128, :] = oc[i * 128:(i + 1) * 128, :]
    return (out, probs)


# revision 7
# speedup vs baseline: 1.1332x; 1.0155x over previous
"""Causal dot-product attention (B=4, S=2048, D=1024, single head) on 8 TRN2 cores.

HW exec time ~570 us/core; scale-relative max-abs error ~3.8e-4 on probs+out.

Sharding: core c = (batch c//2, q-tile parity c%2).  Each core computes the
projections for its batch and attention for its 8 query tiles of 128 rows
(odd or even global q-tiles, descending causal key-need), so the per-slot
key bounds BETA are identical across cores (SPMD-uniform) and causality
skips 44% of the attention FLOPs.  Host gathers query columns per core and
scatters result rows back; the causal mask is an additive per-core input.

Numerics: QxT and the score matmul run in true fp32 (hardware-verified
fp32-accurate; scores have std ~1700 so score error must stay << 1).
KxT uses a 3-pass fp32r hi/lo split (K.T@x ~= Khi@xhi + Khi@xlo + Klo@xhi,
host pre-split; fp32-grade accuracy at 1 cyc/row instead of fp32's 4).
The V path is single-pass fp32r (11-bit mantissa, ~5e-4 relative on `out`).
1/sqrt(d) is folded into the QxT projection copy; softmax runs max/exp with
a fused row-sum (accum_out) and normalizes on-device.

Schedule: P1 QxT (fp32, spilled to DRAM) runs first; P2's pools are created
outside P1 so P2's first slices+K columns prefetch mid-P1 (throttled behind a
buffer-slot wait so startup loads keep full HBM bandwidth); P2 writes KxT to
SBUF direct via 3-pass psum groups; the Vx pool is deferred past P2 to fit
SBUF; attention streams one 128-query slot at a time (scores -> masked
max/exp -> PE-transpose of the unnormalized exp -> exp@V with 1/denom folded
into the output copy, while the normalize-mul + probs DMA run off-path).
"""

import math
import os
from contextlib import ExitStack

import numpy as np

B, S, D = 4, 2048, 1024
NB = D // 128          # 8 contraction chunks
KT = S // 128          # 16 key tiles
NSLOT = 8              # query tiles per core
QL = NSLOT * 128       # local query rows per core (1024)
SCALE = 1.0 / math.sqrt(D)
NEG = -1.0e30

CAUSAL = os.environ.get("KB_CAUSAL", "1") == "1"
USE_FP32R = os.environ.get("KB_FP32R", "1") == "1"
BETA = [16, 14, 12, 10, 8, 6, 4, 2] if CAUSAL else [16] * 8   # k-tiles per slot

_CACHE = {}
LAST_RESULTS = None


def _round_fp32r(a: np.ndarray) -> np.ndarray:
    """Round-to-nearest-even to 11 mantissa bits (the fp32r grid)."""
    b = np.ascontiguousarray(a, dtype=np.float32).view(np.uint32)
    r = (b + np.uint32(0x7FF) + ((b >> np.uint32(12)) & np.uint32(1))) & np.uint32(0xFFFFF000)
    return r.view(np.float32)


def _split_fp32r(a):
    """a ~= hi + lo with both on the fp32r grid (hi 11-bit, lo the residual)."""
    hi = _round_fp32r(a)
    lo = _round_fp32r(np.asarray(a, dtype=np.float32) - hi)
    return hi, lo


def _build_nc():
    import concourse.tile as tile
    from concourse import bacc, mybir
    from concourse.masks import make_identity

    f32 = mybir.dt.float32
    f32r = mybir.dt.float32r
    vdt = f32r if USE_FP32R else f32

    nc = bacc.Bacc("TRN2", target_bir_lowering=False, debug=False)

    xThh = nc.dram_tensor("xThh", [D, S], f32r, kind="ExternalInput")  # hi(xf[b].T)
    xThl = nc.dram_tensor("xThl", [D, S], f32r, kind="ExternalInput")  # lo(xf[b].T)
    xqT = nc.dram_tensor("xqT", [D, QL], f32, kind="ExternalInput")    # query cols of xf[b].T
    xvT = nc.dram_tensor("xvT", [D, S], vdt, kind="ExternalInput")     # x[b].T (rounded)
    Qw = nc.dram_tensor("Qw", [D, D], f32, kind="ExternalInput")
    Kh = nc.dram_tensor("Kh", [D, D], f32r, kind="ExternalInput")
    Kl = nc.dram_tensor("Kl", [D, D], f32r, kind="ExternalInput")
    Vw = nc.dram_tensor("Vw", [D, D], vdt, kind="ExternalInput")       # rounded
    maskT = nc.dram_tensor("maskT", [QL, S], f32, kind="ExternalInput")
    probs_o = nc.dram_tensor("probs_o", [QL, S], f32, kind="ExternalOutput")
    out_o = nc.dram_tensor("out_o", [QL, D], f32, kind="ExternalOutput")

    Exp = mybir.ActivationFunctionType.Exp
    AX = mybir.AxisListType.X

    with tile.TileContext(nc) as tc, ExitStack() as ctx:
        const_pool = ctx.enter_context(tc.tile_pool(name="const", bufs=1))
        ident = const_pool.tile([128, 128], f32)
        make_identity(nc, ident[:])

        kxt_pool = ctx.enter_context(tc.tile_pool(name="kxt", bufs=1))
        kxt = kxt_pool.tile([128, NB * S], f32)          # [d' in chunk, dc*S + k]
        qxtd_pool = ctx.enter_context(tc.tile_pool(name="qxtd", bufs=1, space="DRAM"))
        qxt_d = qxtd_pool.tile([D, QL], f32)

        # ---- P2 pools first so their initial loads run during P1 ----------
        with tc.tile_pool(name="p2k", bufs=2) as p2k, \
             tc.tile_pool(name="p2x", bufs=2) as p2x, \
             tc.tile_pool(name="p2xl", bufs=2) as p2xl:
            xs_tiles = []
            for ks in range(S // 512):
                xsh = p2x.tile([128, NB * 512], f32r, tag="xsh")
                xsl = p2xl.tile([128, NB * 512], f32r, tag="xsl")
                xs_tiles.append((xsh, xsl))
            kc_tiles = []
            for _j in range(2):
                kct = p2k.tile([128, 2 * NB * 128], f32r, tag="kc")
                kc_tiles.append(kct)

            # ---- P1: QxT = (xq @ Q).T * SCALE  (fp32) -> DRAM spill --------
            with tc.tile_pool(name="p1sb", bufs=1) as p1sb, \
                 tc.tile_pool(name="p1q", bufs=2) as p1q, \
                 tc.tile_pool(name="p1st", bufs=1) as p1st, \
                 tc.tile_pool(name="p1ps", bufs=4, space="PSUM") as p1ps:
                xq_all = p1sb.tile([128, NB * QL], f32)
                for qs in range(QL // 512):
                    for dc in range(NB):
                        nc.sync.dma_start(
                            xq_all[:, dc * QL + qs * 512: dc * QL + (qs + 1) * 512],
                            xqT.ap()[dc * 128:(dc + 1) * 128, qs * 512:(qs + 1) * 512])
                for dt_ in range(NB):
                    qcol = p1q.tile([128, NB * 128], f32, tag="qc")
                    nc.sync.dma_start(
                        qcol[:].rearrange("p (c m) -> p c m", c=NB),
                        Qw.ap()[:, dt_ * 128:(dt_ + 1) * 128].rearrange("(c p) m -> p c m", p=128))
                    if dt_ == 3:
                        # prefetch P2's first x-slices + first K columns now --
                        # the qcol slot wait above throttles these behind the
                        # startup-critical loads
                        xsh0, xsl0 = xs_tiles[0]
                        nc.sync.dma_start(
                            xsh0[:].rearrange("p (c m) -> p c m", c=NB),
                            xThh.ap()[:, 0:512].rearrange("(c p) m -> p c m", p=128))
                        nc.sync.dma_start(
                            xsl0[:].rearrange("p (c m) -> p c m", c=NB),
                            xThl.ap()[:, 0:512].rearrange("(c p) m -> p c m", p=128))
                        for j in range(2):
                            nc.sync.dma_start(
                                kc_tiles[j][:, :NB * 128].rearrange("p (c m) -> p c m", c=NB),
                                Kh.ap()[:, j * 128:(j + 1) * 128].rearrange("(c p) m -> p c m", p=128))
                            nc.sync.dma_start(
                                kc_tiles[j][:, NB * 128:].rearrange("p (c m) -> p c m", c=NB),
                                Kl.ap()[:, j * 128:(j + 1) * 128].rearrange("(c p) m -> p c m", p=128))
                    stage = p1st.tile([128, QL], f32)
                    for qs in range(QL // 512):
                        ps = p1ps.tile([128, 512], f32)
                        for dc in range(NB):
                            nc.tensor.matmul(
                                ps[:],
                                qcol[:, dc * 128:(dc + 1) * 128],
                                xq_all[:, dc * QL + qs * 512: dc * QL + (qs + 1) * 512],
                                start=(dc == 0), stop=(dc == NB - 1))
                        nc.scalar.mul(stage[:, qs * 512:(qs + 1) * 512], ps[:], SCALE)
                    nc.gpsimd.dma_start(qxt_d[dt_ * 128:(dt_ + 1) * 128, :], stage[:])

            # ---- P2: KxT = (x @ K).T via 3-pass fp32r hi/lo split ----------
            with tc.tile_pool(name="p2ps", bufs=4, space="PSUM") as p2ps:
                kci = 2
                for ks in range(S // 512):
                    xsh, xsl = xs_tiles[ks]
                    if ks > 0:
                        for g in range(2):
                            nc.sync.dma_start(
                                xsh[:].rearrange("p (c m) -> p c m", c=NB)[:, g * 4:(g + 1) * 4, :],
                                xThh.ap()[g * 512:(g + 1) * 512, ks * 512:(ks + 1) * 512]
                                .rearrange("(c p) m -> p c m", p=128))
                            nc.sync.dma_start(
                                xsl[:].rearrange("p (c m) -> p c m", c=NB)[:, g * 4:(g + 1) * 4, :],
                                xThl.ap()[g * 512:(g + 1) * 512, ks * 512:(ks + 1) * 512]
                                .rearrange("(c p) m -> p c m", p=128))
                    for dt_ in range(NB):
                        if ks == 0 and dt_ < 2:
                            kc_hl = kc_tiles[dt_]
                        else:
                            kc_hl = p2k.tile([128, 2 * NB * 128], f32r, tag="kc")
                            nc.sync.dma_start(
                                kc_hl[:, :NB * 128].rearrange("p (c m) -> p c m", c=NB),
                                Kh.ap()[:, dt_ * 128:(dt_ + 1) * 128].rearrange("(c p) m -> p c m", p=128))
                            nc.sync.dma_start(
                                kc_hl[:, NB * 128:].rearrange("p (c m) -> p c m", c=NB),
                                Kl.ap()[:, dt_ * 128:(dt_ + 1) * 128].rearrange("(c p) m -> p c m", p=128))
                        kcol_h = kc_hl[:, :NB * 128]
                        kcol_l = kc_hl[:, NB * 128:]
                        ps = p2ps.tile([128, 512], f32)
                        i_mm, nmm = 0, 3 * NB
                        for wt, xs in ((kcol_h, xsh), (kcol_h, xsl), (kcol_l, xsh)):
                            for dc in range(NB):
                                nc.tensor.matmul(
                                    ps[:],
                                    wt[:, dc * 128:(dc + 1) * 128],
                                    xs[:, dc * 512:(dc + 1) * 512],
                                    start=(i_mm == 0), stop=(i_mm == nmm - 1))
                                i_mm += 1
                        nc.scalar.copy(kxt[:, dt_ * S + ks * 512: dt_ * S + (ks + 1) * 512], ps[:])

        vx_pool = ctx.enter_context(tc.tile_pool(name="vx", bufs=1))
        vx = vx_pool.tile([128, KT * D], vdt)            # [k in chunk, kc*D + d']

        # ---- P3: Vx = x @ V  (fp32r, resident) -----------------------------
        with tc.tile_pool(name="p3v", bufs=1) as p3v, \
             tc.tile_pool(name="p3x", bufs=2) as p3x, \
             tc.tile_pool(name="p3ps", bufs=4, space="PSUM") as p3ps:
            v_all = p3v.tile([128, NB * D], vdt)
            for dc in range(NB):
                nc.sync.dma_start(v_all[:, dc * D:(dc + 1) * D],
                                  Vw.ap()[dc * 128:(dc + 1) * 128, :])
            for ks in range(S // 512):
                xvs = p3x.tile([128, NB * 512], vdt)
                for dc in range(NB):
                    nc.sync.dma_start(xvs[:, dc * 512:(dc + 1) * 512],
                                      xvT.ap()[dc * 128:(dc + 1) * 128, ks * 512:(ks + 1) * 512])
                for ktl in range(4):
                    kc = ks * 4 + ktl
                    for dsl in range(2):
                        ps = p3ps.tile([128, 512], f32)
                        for dc in range(NB):
                            nc.tensor.matmul(
                                ps[:],
                                xvs[:, dc * 512 + ktl * 128: dc * 512 + (ktl + 1) * 128],
                                v_all[:, dc * D + dsl * 512: dc * D + (dsl + 1) * 512],
                                start=(dc == 0), stop=(dc == NB - 1))
                        nc.scalar.copy(vx[:, kc * D + dsl * 512: kc * D + (dsl + 1) * 512], ps[:])

        # ---- Attention, one slot (128 queries) at a time -------------------
        with tc.tile_pool(name="aq", bufs=2) as aq, \
             tc.tile_pool(name="am", bufs=1) as am, \
             tc.tile_pool(name="asc", bufs=2) as asc, \
             tc.tile_pool(name="apt", bufs=2) as apt, \
             tc.tile_pool(name="aou", bufs=1) as aou, \
             tc.tile_pool(name="ast", bufs=4) as ast, \
             tc.tile_pool(name="aps", bufs=3, space="PSUM") as aps, \
             tc.tile_pool(name="atp", bufs=2, space="PSUM") as atp, \
             tc.tile_pool(name="aop", bufs=2, space="PSUM") as aop:
            for i in range(NSLOT):
                nk = BETA[i]
                kw = nk * 128
                qxt_s = aq.tile([128, NB * 128], f32)
                for dc in range(NB):
                    nc.sync.dma_start(qxt_s[:, dc * 128:(dc + 1) * 128],
                                      qxt_d[dc * 128:(dc + 1) * 128, i * 128:(i + 1) * 128])
                mask_s = am.tile([128, S], f32)
                nc.sync.dma_start(mask_s[:, :kw], maskT.ap()[i * 128:(i + 1) * 128, :kw])

                scores = asc.tile([128, S], f32)
                off = 0
                while off < kw:
                    w = min(512, kw - off)
                    ps = aps.tile([128, 512], f32)
                    for dc in range(NB):
                        nc.tensor.matmul(
                            ps[:, :w],
                            qxt_s[:, dc * 128:(dc + 1) * 128],
                            kxt[:, dc * S + off: dc * S + off + w],
                            start=(dc == 0), stop=(dc == NB - 1))
                    nc.vector.tensor_add(scores[:, off:off + w], ps[:, :w], mask_s[:, off:off + w])
                    off += w

                negm = ast.tile([128, 1], f32)
                nc.vector.reduce_max(negm[:], scores[:, :kw], axis=AX, negate=True)
                probs = scores  # exp in place
                denom = ast.tile([128, 1], f32)
                nc.scalar.activation(probs[:, :kw], scores[:, :kw], Exp,
                                     bias=negm[:], scale=1.0, accum_out=denom[:])
                rec = ast.tile([128, 1], f32)
                nc.vector.reciprocal(rec[:], denom[:])

                # transposes read the unnormalized exp; 1/denom is folded into
                # the PV output copy, so the PE path skips the normalize-mul
                pT = apt.tile([128, S], vdt)
                for kc in range(nk):
                    tps = atp.tile([128, 128], f32)
                    nc.tensor.transpose(tps[:], probs[:, kc * 128:(kc + 1) * 128], ident[:])
                    nc.vector.tensor_copy(pT[:, kc * 128:(kc + 1) * 128], tps[:])

                nc.vector.tensor_scalar_mul(probs[:, :kw], probs[:, :kw], rec[:])
                nc.sync.dma_start(probs_o.ap()[i * 128:(i + 1) * 128, :kw], probs[:, :kw])

                outsb = aou.tile([128, D], f32)
                for dsl in range(2):
                    ps = aop.tile([128, 512], f32)
                    for kc in range(nk):
                        nc.tensor.matmul(
                            ps[:],
                            pT[:, kc * 128:(kc + 1) * 128],
                            vx[:, kc * D + dsl * 512: kc * D + (dsl + 1) * 512],
                            start=(kc == 0), stop=(kc == nk - 1))
                    nc.scalar.activation(outsb[:, dsl * 512:(dsl + 1) * 512], ps[:],
                                         mybir.ActivationFunctionType.Identity,
                                         bias=0.0, scale=rec[:])
                nc.sync.dma_start(out_o.ap()[i * 128:(i + 1) * 128, :], outsb[:])

    nc.compile()
    return nc


def _qtiles(par: int):
    """Global q-tile indices for parity `par`, descending causal need."""
    return [t for t in range(KT - 1, -1, -1) if t % 2 == par]


def kernel(x, Q, K, V, F):
    global LAST_RESULTS
    try:
        from antenv.axon_hooks import get_axon_ntff_profile_hook  # noqa: F401
    except Exception:
        # tracing would crash without the axon NTFF hook module
        os.environ["BASS_NEVER_TRACE"] = "1"
    from concourse.bass_utils import run_bass_kernel_spmd

    x = np.ascontiguousarray(x, dtype=np.float32)
    Q = np.ascontiguousarray(Q, dtype=np.float32)
    K = np.ascontiguousarray(K, dtype=np.float32)
    V = np.ascontiguousarray(V, dtype=np.float32)
    F = np.ascontiguousarray(F, dtype=np.float32)

    if np.array_equal(F, np.eye(D, dtype=np.float32)):
        xf = x
    else:
        xf = np.matmul(x, F)

    if "nc" not in _CACHE:
        _CACHE["nc"] = _build_nc()
    nc = _CACHE["nc"]

    Vr = _round_fp32r(V) if USE_FP32R else V
    Kh_, Kl_ = _split_fp32r(K)

    # host-side per-core tensors
    xT_b = [np.ascontiguousarray(xf[b].T) for b in range(B)]
    xTs_b = [_split_fp32r(xT_b[b]) for b in range(B)]
    xvT_b = [_round_fp32r(x[b].T) if USE_FP32R else np.ascontiguousarray(x[b].T)
             for b in range(B)]

    kpos = np.arange(S, dtype=np.int64)
    masks, qsels = {}, {}
    for par in (0, 1):
        tiles = _qtiles(par)
        qpos = np.concatenate([np.arange(t * 128, (t + 1) * 128, dtype=np.int64)
                               for t in tiles])
        masks[par] = np.where(kpos[None, :] <= qpos[:, None],
                              np.float32(0.0), np.float32(NEG)).astype(np.float32)
        qsels[par] = qpos

    in_maps = []
    for c in range(8):
        b, par = c // 2, c % 2
        in_maps.append({
            "xThh": xTs_b[b][0], "xThl": xTs_b[b][1],
            "xqT": np.ascontiguousarray(xT_b[b][:, qsels[par]]),
            "xvT": xvT_b[b],
            "Qw": Q, "Kh": Kh_, "Kl": Kl_, "Vw": Vr,
            "maskT": masks[par],
        })

    res = run_bass_kernel_spmd(nc, in_maps, list(range(8)))
    LAST_RESULTS = res

    probs = np.zeros((B, S, S), dtype=np.float32)
    out = np.zeros((B, S, D), dtype=np.float32)
    for c in range(8):
        b, par = c // 2, c % 2
        tiles = _qtiles(par)
        pc = res.results[c]["probs_o"]
        oc = res.results[c]["out_o"]
        for i, t in enumerate(tiles):
            probs[b, t * 128:(t + 1) * 128, :] = pc[i * 128:(i + 1) * 128, :]
            out[b, t * 128:(t + 1) * 128, :] = oc[i * 128:(i + 1) * 128, :]
    return (out, probs)
